# revision 31
# baseline (speedup 1.0000x reference)
"""TreeLSTM-style CARD module kernel for 8 Trainium2 NeuronCores.

Strategy (per sharding hint): data-parallel over whole trees — 64 complete
binary trees (511 nodes each, heap order) are split 8 trees/core. On the host,
each core's nodes are re-ordered LEVEL-MAJOR (all leaves of its 8 trees first,
then level 1, ... root level 8) and all per-node features are TRANSPOSED to
feature-major [F, cols]. With that layout:
  * every matmul keeps the contraction dim on SBUF partitions
    (weights are the stationary lhsT operand, activations the moving rhs),
  * the child-sum of the tree recurrence becomes a contiguous stride-2
    pair-sum along the SBUF free dimension (heap layout ⇒ children of the
    whole level block are adjacent pairs of the previous level block),
  * all gathers/scatters disappear; weights are replicated per core.
Matmuls run as float32r (full PE rate at moving-dim 512). Outputs come back
feature-major and are transposed/un-permuted on the host.
"""

import numpy as np

import concourse.bass as bass
import concourse.tile as tile
from concourse import mybir
from concourse.bass_utils import run_bass_kernel_spmd

F32 = mybir.dt.float32
F32R = mybir.dt.float32r
AF = mybir.ActivationFunctionType

# ---------------------------------------------------------------- shapes
D = 8
T = 64
PER_TREE = 2 ** (D + 1) - 1          # 511
N = T * PER_TREE                     # 32704
NCORES = 8
TPC = T // NCORES                    # 8 trees per core
VCOLS = TPC * PER_TREE               # 4088 valid cols per core
PCOLS = 4096                         # padded cols per core
CHUNK = 512
NCHUNK = PCOLS // CHUNK              # 8
INPUT_DIM = 128
MEM = 5 * INPUT_DIM                  # 640
HID = 512
FEATS = [("op", 32), ("tb", 64), ("ft", 256), ("jn", 128), ("cd", 16)]

# level n (0 = leaves .. 8 = root) block sizes / starts in per-core col space
LSIZE = [TPC * (2 ** (D - n)) for n in range(D + 1)]    # [2048,1024,...,8]
LSTART = [0]
for s in LSIZE:
    LSTART.append(LSTART[-1] + s)                        # LSTART[9] == 4088

# c lives in a ring of 3072 cols (level n-1 is dead once level n+1 starts)
CRING = 3072
RSTART = [0] * (D + 1)
for n in range(1, D + 1):
    RSTART[n] = (RSTART[n - 1] + LSIZE[n - 1]) % CRING
# RSTART == [0,2048,0,512,768,896,960,992,1008]; levels 3..8 end at 1016
RING_OF_CHUNK = [0, 512, 1024, 1536, 2048, 2560, 0, 512]


def _node_to_col():
    """col index (within a core, tree-local t) for heap node r of tree t."""
    r = np.arange(PER_TREE)
    d = np.floor(np.log2(r + 1)).astype(np.int64)        # depth, 0 at root
    n = D - d                                            # level
    i = r - (2 ** d - 1)
    cols = np.empty((TPC, PER_TREE), dtype=np.int64)
    for t in range(TPC):
        cols[t] = np.asarray(LSTART)[n] + t * (2 ** (D - n)) + i
    return cols.reshape(-1)                              # [4088]


NODE_TO_COL = _node_to_col()

_NC_CACHE = {}


def _jobs_for_chunk(j):
    """(level, global col range [a,b)) pieces contained in chunk j."""
    J = j * CHUNK
    out = []
    for n in range(D + 1):
        a, b = max(LSTART[n], J), min(LSTART[n + 1], J + CHUNK)
        if a < b:
            out.append((n, a, b))
    return out


def _hoist_matmul_waits(nc):
    """Fused-LW f32r matmuls encode at most ONE sync-wait in the ISA Events
    struct and walrus refuses to split multi-waits on that path. Hoist every
    matmul's extra waits onto a NoOp inserted just before it on the same
    engine queue (identical semantics: the queue stalls on the NoOp's waits
    before the matmul issues)."""
    for blk in nc.m.functions[0].blocks:
        out = []
        for ins in blk.instructions:
            si = ins.sync_info
            if si is not None and si.on_wait and len(si.on_wait) > 1:
                for w in si.on_wait[:-1]:
                    nop = mybir.InstNoOp(
                        name=nc.get_next_instruction_name(),
                        engine=ins.engine,
                        sync_info=mybir.SyncInfo(on_wait=[w], on_update=[]),
                        bass_nofuse=True,
                    )
                    out.append(nop)
                ins.sync_info = mybir.SyncInfo(
                    on_wait=[si.on_wait[-1]], on_update=list(si.on_update))
            out.append(ins)
        blk.instructions = out


def _build_nc():
    nc = bass.Bass()
    X = {}
    for nm, dim in FEATS:
        X[nm] = nc.dram_tensor(nm + "T", [dim, PCOLS], F32R, kind="ExternalInput")
        X["w" + nm + "1"] = nc.dram_tensor("w_" + nm + "1", [dim, 128], F32R,
                                           kind="ExternalInput")
        X["w" + nm + "2"] = nc.dram_tensor("w_" + nm + "2", [128, 128], F32R,
                                           kind="ExternalInput")
    X["b1"] = nc.dram_tensor("b1", [128, 5], F32, kind="ExternalInput")
    X["b2"] = nc.dram_tensor("b2", [128, 5], F32, kind="ExternalInput")
    X["wx"] = nc.dram_tensor("w_xou", [MEM, 3 * MEM], F32R, kind="ExternalInput")
    X["bx"] = nc.dram_tensor("b_xou", [128, 15], F32, kind="ExternalInput")
    X["bxn"] = nc.dram_tensor("b_xou_neg", [128, 15], F32, kind="ExternalInput")
    X["wo1"] = nc.dram_tensor("w_o1", [MEM, HID], F32R, kind="ExternalInput")
    X["bo1"] = nc.dram_tensor("b_o1", [128, 4], F32, kind="ExternalInput")
    X["wo2"] = nc.dram_tensor("w_o2", [HID, 1], F32R, kind="ExternalInput")
    X["bo2"] = nc.dram_tensor("b_o2", [1, 1], F32, kind="ExternalInput")
    c_out = nc.dram_tensor("c_out", [MEM, PCOLS], F32, kind="ExternalOutput")
    o_out = nc.dram_tensor("o_out", [1, PCOLS], F32, kind="ExternalOutput")

    with tile.TileContext(nc) as tc:
        with tc.tile_pool(name="w", bufs=1) as wp, \
             tc.tile_pool(name="cst", bufs=1) as cp, \
             tc.tile_pool(name="work", bufs=1) as kp, \
             tc.tile_pool(name="ps", bufs=7, space="PSUM") as pp, \
             tc.tile_pool(name="pso", bufs=1, space="PSUM") as pp1:

            # ---------------- resident weights
            w1 = {}
            for nm, dim in FEATS:
                if dim <= 128:
                    t_ = wp.tile([dim, 128], F32R, name="w1" + nm)
                    nc.sync.dma_start(out=t_, in_=X["w" + nm + "1"][:, :])
                    w1[nm] = [t_]
                else:
                    parts = []
                    for k in range(dim // 128):
                        t_ = wp.tile([128, 128], F32R, name=f"w1{nm}{k}")
                        nc.sync.dma_start(
                            out=t_, in_=X["w" + nm + "1"][k * 128:(k + 1) * 128, :])
                        parts.append(t_)
                    w1[nm] = parts
            w2 = {}
            for nm, _ in FEATS:
                t_ = wp.tile([128, 128], F32R, name="w2" + nm)
                nc.sync.dma_start(out=t_, in_=X["w" + nm + "2"][:, :])
                w2[nm] = t_
            # big weights (wx/wo1/wo2): tiles made here, DMAs emitted inside
            # chunk 0 after its input loads so the first MLPs start promptly
            wx = [wp.tile([128, 3 * MEM], F32R, name=f"wx{k}") for k in range(5)]
            wo1 = [wp.tile([128, HID], F32R, name=f"wo1{k}") for k in range(5)]
            wo2 = [wp.tile([128, 1], F32R, name=f"wo2{g}") for g in range(4)]

            def load_big_weights():
                # halves interleaved so consecutive descriptors land on
                # different HWDGE queues and the 4.9MB w_xou streams in
                # parallel instead of serializing ahead of the first xou
                for k in range(5):
                    nc.sync.dma_start(out=wx[k][:, :960],
                                      in_=X["wx"][k * 128:(k + 1) * 128, :960])
                    nc.sync.dma_start(out=wx[k][:, 960:],
                                      in_=X["wx"][k * 128:(k + 1) * 128, 960:])
                for k in range(5):
                    nc.sync.dma_start(out=wo1[k],
                                      in_=X["wo1"][k * 128:(k + 1) * 128, :])
                for g in range(4):
                    nc.sync.dma_start(out=wo2[g],
                                      in_=X["wo2"][g * 128:(g + 1) * 128, :])

            b1 = wp.tile([128, 5], F32, name="b1t")
            nc.sync.dma_start(out=b1, in_=X["b1"][:, :])
            b2 = wp.tile([128, 5], F32, name="b2t")
            nc.sync.dma_start(out=b2, in_=X["b2"][:, :])
            bx = wp.tile([128, 15], F32, name="bxt")
            nc.sync.dma_start(out=bx, in_=X["bx"][:, :])
            bxn = wp.tile([128, 15], F32, name="bxnt")
            nc.sync.dma_start(out=bxn, in_=X["bxn"][:, :])
            bo1 = wp.tile([128, 4], F32, name="bo1t")
            nc.sync.dma_start(out=bo1, in_=X["bo1"][:, :])
            bo2 = wp.tile([1, 1], F32, name="bo2t")
            nc.sync.dma_start(out=bo2, in_=X["bo2"][:, :])

            # ---------------- resident c ring (5 feature chunks x 3072 cols)
            cring = [cp.tile([128, CRING], F32, name=f"cring{f}") for f in range(5)]
            for f in range(5):
                nc.gpsimd.memset(cring[f][:, 1016:1024], 0.0)  # chunk-7 DMA pad

            r = lambda ap: ap if ap.dtype == F32R else ap.bitcast(F32R)

            def pe_touch(*aps):
                # PE no-op that reads the given tiles: Tile hangs the cross-
                # engine waits here so fused-LW f32r matmuls carry <=1 wait
                # (the ISA Events struct encodes a single wait condition and
                # walrus refuses to split waits on the LW path). APs are
                # stripped from InstNoOp at Tile lowering.
                for ap in aps:
                    inst = mybir.InstNoOp(
                        name=nc.get_next_instruction_name(),
                        ins=[nc.tensor.lower_ap(ap)],
                        outs=[],
                        bass_nofuse=True,
                    )
                    nc.tensor.add_instruction(inst)

            # ---------------- chunk pipeline
            for j in range(NCHUNK):
                J = j * CHUNK
                leaf = j < 4
                # ---- load inputs
                inp = kp.tile([32, CHUNK], F32R, tag="in_op", bufs=1,
                              name=f"inop{j}")
                nc.sync.dma_start(out=inp, in_=X["op"][:, J:J + CHUNK])
                itb = kp.tile([64, CHUNK], F32R, tag="in_tb", bufs=1,
                              name=f"intb{j}")
                nc.sync.dma_start(out=itb, in_=X["tb"][:, J:J + CHUNK])
                icd = kp.tile([16, CHUNK], F32R, tag="in_cd", bufs=1,
                              name=f"incd{j}")
                nc.sync.dma_start(out=icd, in_=X["cd"][:, J:J + CHUNK])
                ft0 = kp.tile([128, CHUNK], F32R, tag="in_ft0", bufs=1,
                              name=f"ft0_{j}")
                nc.sync.dma_start(out=ft0, in_=X["ft"][0:128, J:J + CHUNK])
                ft1 = kp.tile([128, CHUNK], F32R, tag="in_ft1", bufs=1,
                              name=f"ft1_{j}")
                nc.sync.dma_start(out=ft1, in_=X["ft"][128:256, J:J + CHUNK])
                jn = kp.tile([128, CHUNK], F32R, tag="in_jn", bufs=1,
                              name=f"jn{j}")
                nc.sync.dma_start(out=jn, in_=X["jn"][:, J:J + CHUNK])
                if j == 0:
                    load_big_weights()
                    for wt in ([t for ps_ in w1.values() for t in ps_]
                               + list(w2.values()) + wx + wo1 + wo2):
                        pe_touch(wt[:, :])
                rhs1 = {"op": [inp], "tb": [itb], "cd": [icd],
                        "ft": [ft0, ft1], "jn": [jn]}
                pe_touch(inp[:, :], itb[:, :], icd[:, :], ft0[:, :], ft1[:, :],
                         jn[:, :])

                # ---- per-type 2-layer MLPs -> xT (feature-major x)
                xT = []
                for f, (nm, dim) in enumerate(FEATS):
                    ps = pp.tile([128, CHUNK], F32, tag="ps", name=f"psA{j}{nm}")
                    for k, rhs_k in enumerate(rhs1[nm]):
                        nc.tensor.matmul(ps[:, :], r(w1[nm][k][:, :]), r(rhs_k),
                                         start=(k == 0), stop=(k == len(rhs1[nm]) - 1))
                    h1 = kp.tile([128, CHUNK], F32R, tag="h1", bufs=3,
                                 name=f"h1{j}{nm}")
                    # relu(psum + b) on DVE: (in + b) max 0
                    nc.vector.tensor_scalar(h1[:, :], ps[:, :], b1[:, f:f + 1],
                                            0.0, mybir.AluOpType.add,
                                            mybir.AluOpType.max)
                    pe_touch(h1[:, :])
                    ps2 = pp.tile([128, CHUNK], F32, tag="ps", name=f"psB{j}{nm}")
                    nc.tensor.matmul(ps2[:, :], r(w2[nm][:, :]), r(h1[:, :]),
                                     start=True, stop=True)
                    xt = kp.tile([128, CHUNK], F32R, tag=f"xT_{f}", bufs=2,
                                 name=f"xT{j}{nm}")
                    nc.scalar.activation(xt[:, :], ps2[:, :], AF.Relu,
                                         bias=b2[:, f:f + 1])
                    pe_touch(xt[:, :])
                    xT.append(xt)

                # ---- xou = x @ w_xou (+b): xx | ff | rr blocks, feature-major
                def xou_pass(m, ks, ps=None):
                    if ps is None:
                        ps = pp.tile([128, CHUNK], F32, tag="ps",
                                     name=f"psX{j}_{m}")
                    for k in ks:
                        nc.tensor.matmul(ps[:, :],
                                         r(wx[k][:, m * 128:(m + 1) * 128]),
                                         r(xT[k][:, :]),
                                         start=(k == 0), stop=(k == 4))
                    return ps

                def xou_psum(m):
                    return xou_pass(m, range(5))

                xx, fg, rr, rc = [], [], [], []
                if leaf:
                    # per f: ff psum -> 1-sigmoid evict, xx psum ->
                    # c = (psum+b)*(1-ff) in one DVE op (no psum pile-up)
                    ff_head = {}
                    if j == 0:
                        # emit the f-gate k=0..2 partials first: they only
                        # need wx[0:3], so the PE works while the rest of
                        # w_xou is still streaming in
                        for f in range(5):
                            ff_head[f] = xou_pass(5 + f, range(0, 3))
                    for f in range(5):
                        if f in ff_head:
                            ps = xou_pass(5 + f, range(3, 5), ps=ff_head[f])
                        else:
                            ps = xou_psum(5 + f)
                        t_ = kp.tile([128, CHUNK], F32, tag=f"fg_{f}", bufs=1,
                                     name=f"fg{j}_{f}")
                        nc.scalar.activation(t_[:, :].bitcast(F32R), ps[:, :],
                                             AF.Sigmoid,
                                             bias=bxn[:, 5 + f:6 + f], scale=-1.0)
                        fg.append(t_)
                        ps_xx = xou_psum(f)
                        cs = cring[f][:, J:J + CHUNK]
                        nc.vector.scalar_tensor_tensor(
                            cs, ps_xx[:, :], bx[:, f:f + 1], t_[:, :],
                            mybir.AluOpType.add, mybir.AluOpType.mult)
                else:
                    for f in range(5):
                        ps = xou_psum(f)
                        t_ = kp.tile([128, CHUNK], F32, tag=f"xx_{f}", bufs=1,
                                     name=f"xx{j}_{f}")
                        # ACT (Identity+bias) — DVE is the busier engine in
                        # the nonleaf chunks and PSUM reads cost it 2x mode
                        nc.scalar.activation(t_[:, :], ps[:, :], AF.Identity,
                                             bias=bx[:, f:f + 1])
                        xx.append(t_)
                    for f in range(5):
                        ps = xou_psum(5 + f)
                        t_ = kp.tile([128, CHUNK], F32, tag=f"fg_{f}", bufs=1,
                                     name=f"fg{j}_{f}")
                        nc.scalar.activation(t_[:, :].bitcast(F32R), ps[:, :],
                                             AF.Sigmoid,
                                             bias=bx[:, 5 + f:6 + f])
                        fg.append(t_)
                for f in range(5):
                    ps = xou_psum(10 + f)
                    t_ = kp.tile([128, CHUNK], F32, tag=f"rr_{f}", bufs=1,
                                 name=f"rr{j}_{f}")
                    nc.scalar.activation(t_[:, :], ps[:, :], AF.Sigmoid,
                                         bias=bx[:, 10 + f:11 + f])
                    rr.append(t_)
                    t2 = kp.tile([128, CHUNK], F32, tag=f"rc_{f}", bufs=1,
                                 name=f"rc{j}_{f}")
                    nc.scalar.activation(t2[:, :], ps[:, :], AF.Sigmoid,
                                         bias=bxn[:, 10 + f:11 + f], scale=-1.0)
                    rc.append(t2)

                # ---- tree recurrence; th and then h land in the fg tiles
                h = fg

                def c_path(n, a, b, f):
                    la, sz = a - J, b - a
                    rs_ = RSTART[n] + (a - LSTART[n])      # c ring cols of [a,b)
                    cs = cring[f][:, rs_:rs_ + sz]
                    if n > 0:
                        ks = RSTART[n - 1] + 2 * (a - LSTART[n])
                        kv = cring[f][:, ks:ks + 2 * sz].rearrange(
                            "p (m two) -> p m two", two=2)
                        # c = ff*(pairsum - xx) + xx, in-place in c
                        nc.vector.tensor_add(cs, kv[:, :, 0], kv[:, :, 1])
                        nc.vector.tensor_sub(cs, cs, xx[f][:, la:la + sz])
                        nc.vector.tensor_mul(cs, fg[f][:, la:la + sz], cs)
                        nc.vector.tensor_add(cs, xx[f][:, la:la + sz], cs)
                    return cs

                def h_path(n, a, b, f, small):
                    la, sz = a - J, b - a
                    rs_ = RSTART[n] + (a - LSTART[n])
                    cs = cring[f][:, rs_:rs_ + sz]
                    # h = rr*tanh(c) + (1-rr)*x  (th->fg, m1->rr, m2->rc)
                    nc.scalar.activation(fg[f][:, la:la + sz].bitcast(F32R), cs,
                                         AF.Tanh)
                    nc.vector.tensor_mul(rr[f][:, la:la + sz],
                                         rr[f][:, la:la + sz],
                                         fg[f][:, la:la + sz])
                    m2e = nc.vector if small else nc.gpsimd
                    m2e.tensor_mul(rc[f][:, la:la + sz],
                                   rc[f][:, la:la + sz],
                                   xT[f][:, la:la + sz].bitcast(F32))
                    nc.vector.tensor_add(
                        h[f][:, la:la + sz].bitcast(F32R),
                        rr[f][:, la:la + sz], rc[f][:, la:la + sz])

                def recurrence(jobs, c_first=False):
                    if c_first:
                        # emit the level->level critical chain (all DVE) ahead
                        # of the off-path h computation
                        for (n, a, b) in jobs:
                            for f in range(5):
                                c_path(n, a, b, f)
                        for (n, a, b) in jobs:
                            for f in range(5):
                                h_path(n, a, b, f, small=True)
                    else:
                        for (n, a, b) in jobs:
                            for f in range(5):
                                c_path(n, a, b, f)
                                h_path(n, a, b, f, small=False)

                # out = sigmoid(relu(h@w_o1+b) @ w_o2 + b) for cols [lo:hi)
                ob = kp.tile([1, CHUNK], F32, tag="outs", bufs=2, name=f"ob{j}")

                def out_mlp(lo, hi):
                    w = hi - lo
                    for k in range(5):
                        pe_touch(h[k][:, lo:hi])
                    o1T = []
                    for g in range(4):
                        ps = pp.tile([128, w], F32, tag="ps",
                                     name=f"psO{j}_{g}_{lo}")
                        for k in range(5):
                            nc.tensor.matmul(ps[:, :],
                                             r(wo1[k][:, g * 128:(g + 1) * 128]),
                                             r(h[k][:, lo:hi]),
                                             start=(k == 0), stop=(k == 4))
                        t_ = kp.tile([128, w], F32R, tag="o1T", bufs=3,
                                     name=f"o1T{j}_{g}_{lo}")
                        nc.scalar.activation(t_[:, :], ps[:, :], AF.Relu,
                                             bias=bo1[:, g:g + 1])
                        pe_touch(t_[:, :])
                        o1T.append(t_)
                    ps1 = pp1.tile([1, w], F32, tag="pso", name=f"psZ{j}_{lo}")
                    for g in range(4):
                        nc.tensor.matmul(ps1[:, :], r(wo2[g][:, :]),
                                         r(o1T[g][:, :]),
                                         start=(g == 0), stop=(g == 3))
                    nc.scalar.activation(ob[:, lo:hi], ps1[:, :], AF.Sigmoid,
                                         bias=bo2[:, 0:1])

                jobs = _jobs_for_chunk(j)
                if j == NCHUNK - 1:
                    # pipeline the serial tail: level 3 -> its out MLP while
                    # levels 4..8 recur -> rest of out MLP
                    recurrence(jobs[:1])
                    out_mlp(0, 256)
                    recurrence(jobs[1:], c_first=True)
                    out_mlp(256, CHUNK)
                else:
                    recurrence(jobs)
                    out_mlp(0, CHUNK)

                # ---- store c and out for this chunk
                rs = RING_OF_CHUNK[j]
                for f in range(5):
                    nc.sync.dma_start(
                        out=c_out[f * 128:(f + 1) * 128, J:J + CHUNK],
                        in_=cring[f][:, rs:rs + CHUNK])
                nc.sync.dma_start(out=o_out[0:1, J:J + CHUNK], in_=ob[:, :])

    _hoist_matmul_waits(nc)
    return nc


def _get_nc():
    if "nc" not in _NC_CACHE:
        _NC_CACHE["nc"] = _build_nc()
    return _NC_CACHE["nc"]


def _prep_in_maps(inputs):
    f32 = np.float32
    g = {k: np.asarray(v) for k, v in inputs.items()}

    # global scatter: node (core,t,r) -> col core*PCOLS + NODE_TO_COL[t*511+r]
    gcol = (np.arange(NCORES)[:, None] * PCOLS + NODE_TO_COL[None, :]).reshape(-1)

    featT = {}
    for nm, dim in FEATS:
        key = {"op": "op_feat", "tb": "tb_feat", "ft": "ft_feat",
               "jn": "join_feat", "cd": "card_feat"}[nm]
        big = np.zeros((dim, NCORES * PCOLS), dtype=f32)
        big[:, gcol] = np.ascontiguousarray(g[key].astype(f32, copy=False).T)
        featT[nm] = big

    bx = np.ascontiguousarray(g["b_xou"].astype(f32).reshape(15, 128).T)
    shared = {
        "b1": np.stack([g["b_" + nm + "1"] for nm, _ in FEATS], 1).astype(f32),
        "b2": np.stack([g["b_" + nm + "2"] for nm, _ in FEATS], 1).astype(f32),
        "w_xou": np.ascontiguousarray(g["w_xou"].astype(f32)),
        "b_xou": bx,
        "b_xou_neg": np.ascontiguousarray(-bx),
        "w_o1": np.ascontiguousarray(g["w_o1"].astype(f32)),
        "b_o1": np.ascontiguousarray(g["b_o1"].astype(f32).reshape(4, 128).T),
        "w_o2": np.ascontiguousarray(g["w_o2"].astype(f32)),
        "b_o2": g["b_o2"].astype(f32).reshape(1, 1),
    }
    for nm, _ in FEATS:
        shared["w_" + nm + "1"] = np.ascontiguousarray(g["w_" + nm + "1"].astype(f32))
        shared["w_" + nm + "2"] = np.ascontiguousarray(g["w_" + nm + "2"].astype(f32))

    in_maps = []
    for c in range(NCORES):
        m = dict(shared)
        for nm, _ in FEATS:
            m[nm + "T"] = np.ascontiguousarray(
                featT[nm][:, c * PCOLS:(c + 1) * PCOLS])
        in_maps.append(m)
    return in_maps


def kernel(**inputs):
    import importlib
    import os

    if os.environ.get("BASS_TRACE") and not os.environ.get("BASS_NEVER_TRACE"):
        try:
            importlib.import_module("antenv.axon_hooks")
        except ImportError:
            # axon NTFF hook missing in this client; tracing would crash
            os.environ["BASS_NEVER_TRACE"] = "1"

    nc = _get_nc()
    in_maps = _prep_in_maps(inputs)
    br = run_bass_kernel_spmd(nc, in_maps, core_ids=list(range(NCORES)))
    kernel.last_results = br

    out = np.empty((N, 1), dtype=np.float32)
    c = np.empty((N, MEM), dtype=np.float32)
    for core in range(NCORES):
        res = br.results[core]
        sl = slice(core * VCOLS, (core + 1) * VCOLS)
        c[sl] = res["c_out"][:, NODE_TO_COL].T
        out[sl, 0] = res["o_out"][0, NODE_TO_COL]
    return out, c
